# revision 1
# baseline (speedup 1.0000x reference)
"""Trainium2 Bass kernel for ConvBnSign (binarized 3x3 conv + sync-BN + sign).

Math: y = conv2d(x, sign(w) * alpha)  with alpha = mean|w| per out-channel,
then train-mode BatchNorm over (N,H,W), then hard_sign.

Since alpha_o > 0 is a per-channel scale, fold it into the BN affine:
  z = conv2d(x, sign(w))          (exact +-1 weights -> exact in bf16)
  y = alpha * z; mean_y = alpha*mu_z; var_y = alpha^2*var_z
  out = sign((z - mu_z) * A + beta)  with A = alpha*gamma*rsqrt(alpha^2 var_z + eps)
      = sign(z*A + B),  B = beta - mu_z*A

Precision: x is split on host into bf16 hi + lo (combined ~2^-18 relative);
each 3x3 tap is two accumulating bf16 matmuls into fp32 PSUM.

Sharding: data-parallel, 4 images per core across 8 cores; BN stats are
per-channel partial sums [128,4] fp32 all-reduced across cores.
"""

import numpy as np
import ml_dtypes

import concourse.bass as bass
import concourse.mybir as mybir
import concourse.tile as tile
from concourse.vector_clock import ScopedClock
from concourse.bass_utils import run_bass_kernel_spmd

# ---- problem constants (hardcoded per contract) ----
N_CORES = 8
N_FULL = 32           # batch
CIN = 128             # input channels
COUT = 256            # output channels
H = W = 56
KH = KW = 3
BN_EPS = 1e-5

IMGS = N_FULL // N_CORES          # 4 images per core
WP = W + 2                        # 58 padded width
HP = H + 2
PADPIX = HP * WP                  # 3364
PIX = H * W                       # 3136
NCHUNK = COUT // 128              # 2 chunks of 128 output channels
RTR = 8                           # rows per matmul tile
RT = H // RTR                     # 7 row tiles per image
NTILE = RTR * W                   # 448 = matmul free dim (<=512, one PSUM bank)
NTOT = N_FULL * PIX               # 200704 elements per channel for BN stats

BF16 = mybir.dt.bfloat16
F32 = mybir.dt.float32

_MAX_DRAIN_WAITS = 1  # walrus CTRL instructions accept a single sync wait


def _split_multi_waits(nc, max_waits=1):
    """This walrus build rejects instructions with more than one sem wait.
    Hoist excess waits onto same-engine NoOps inserted immediately before the
    offending instruction (the engine blocks at the NoOp instead — identical
    ordering semantics)."""
    ctr = 0
    for bbw in nc.main_func.blocks:
        out = []
        changed = False
        for inst in bbw.instructions:
            si = inst.sync_info
            w = list(si.on_wait or []) if si else []
            if len(w) > max_waits:
                changed = True
                excess = w[: len(w) - max_waits]
                for i in range(0, len(excess), max_waits):
                    nop = mybir.InstNoOp(name=f"WFIX-{ctr}", ins=[], outs=[])
                    ctr += 1
                    nop.engine = inst.engine
                    nop.sync_info = mybir.SyncInfo(
                        on_wait=excess[i : i + max_waits], on_update=[]
                    )
                    out.append(nop)
                inst.sync_info = mybir.SyncInfo(
                    on_wait=w[len(w) - max_waits :],
                    on_update=list(si.on_update or []),
                )
            out.append(inst)
        if changed:
            bbw.instructions = out
    return ctr


class _SplitDrainTileContext(tile.TileContext):
    """TileContext whose final drain splits its sem waits across multiple
    sync-engine instructions (this walrus build caps CTRL waits at 1)."""

    def _drain_and_barrier(self, tick_clock, wait_clock):
        drain_inst = self.nc.sync.drain()
        wait_clock.add_sem_waits(
            drain_inst.ins, ScopedClock({None: tick_clock.global_clock})
        )
        si = drain_inst.ins.sync_info
        w = list(si.on_wait or [])
        if len(w) > _MAX_DRAIN_WAITS:
            drain_inst.ins.sync_info = mybir.SyncInfo(
                on_wait=w[:_MAX_DRAIN_WAITS], on_update=list(si.on_update or [])
            )
            for i in range(_MAX_DRAIN_WAITS, len(w), _MAX_DRAIN_WAITS):
                nop = self.nc.sync.nop(nofuse=True)
                nop.ins.sync_info = mybir.SyncInfo(
                    on_wait=w[i : i + _MAX_DRAIN_WAITS], on_update=[]
                )
        self.nc.all_engine_barrier()
        assert self.sems is not None
        popped = self.nc._tile_sem_poison_stack.pop()
        assert popped is self._sem_poison
        self.nc.clear_and_free_semaphores(list(self.sems.allocated().values()))
        self.nc.all_engine_barrier()


def build_bass(n_cores=N_CORES, collective=True):
    """Build the per-core Bass module (SPMD: same program on every core)."""
    nc = bass.Bass(num_devices=n_cores)

    xh_d = nc.dram_tensor("xh", [IMGS, CIN, PADPIX], BF16, kind="ExternalInput")
    xl_d = nc.dram_tensor("xl", [IMGS, CIN, PADPIX], BF16, kind="ExternalInput")
    ws_d = nc.dram_tensor("ws", [CIN, KH * KW * COUT], BF16, kind="ExternalInput")
    abg_d = nc.dram_tensor("abg", [128, 3 * NCHUNK], F32, kind="ExternalInput")
    out_d = nc.dram_tensor("out", [IMGS, NCHUNK, 128, PIX], BF16,
                           kind="ExternalOutput")

    with _SplitDrainTileContext(nc) as tc:
        with (
            tc.tile_pool(name="const", bufs=1) as constp,
            tc.tile_pool(name="xbuf", bufs=1) as xp,
            tc.tile_pool(name="zbuf", bufs=1) as zp,
            tc.tile_pool(name="stats", bufs=1) as sp,
            tc.tile_pool(name="sq", bufs=2) as sqp,
            tc.tile_pool(name="pz", bufs=8, space="PSUM") as pp,
            tc.tile_pool(name="dram", bufs=1, space="DRAM") as dp,
        ):
            # ---- constants ----
            w_sb = constp.tile([128, KH * KW * COUT], BF16, tag="wsgn")
            abg_sb = constp.tile([128, 3 * NCHUNK], F32, tag="abg")
            nc.sync.dma_start(w_sb[:], ws_d[:])
            nc.sync.dma_start(abg_sb[:], abg_d[:])
            w_v = w_sb[:].rearrange("p (k o) -> p k o", k=KH * KW)

            # ---- x tiles (per image, hi/lo) ----
            xt = {}
            for img in range(IMGS):
                for half, src in (("h", xh_d), ("l", xl_d)):
                    t = xp.tile([128, PADPIX], BF16, tag=f"x{half}{img}", name=f"x{half}{img}")
                    nc.sync.dma_start(t[:], src[img])
                    xt[(half, img)] = t

            # ---- z buffers + stats ----
            z = [zp.tile([128, IMGS * PIX], F32, tag=f"z{j}", name=f"z{j}")
                 for j in range(NCHUNK)]
            ssum = sp.tile([128, 64], F32, tag="ssum")
            ssq = sp.tile([128, 64], F32, tag="ssq")

            alpha = abg_sb[:, 0:NCHUNK]
            gamma = abg_sb[:, NCHUNK : 2 * NCHUNK]
            beta = abg_sb[:, 2 * NCHUNK : 3 * NCHUNK]
            inv_n = 1.0 / NTOT
            npart = IMGS * RT

            # hi taps first: PE can start before any x_lo has arrived
            taps = [(k, "h") for k in range(KH * KW)] + \
                   [(k, "l") for k in range(KH * KW)]

            # Per chunk: conv -> stats AllReduce -> sign+store. Chunk 0's
            # collective + BN tail overlaps chunk 1's conv on PE.
            for j in range(NCHUNK):
                for img in range(IMGS):
                    # tile-major: one PSUM tile completes every 18 matmuls, so
                    # bank releases interleave smoothly with the next tile's
                    # compute (ldw-opt is off, so weight-major saved nothing)
                    for rt in range(RT):
                        pt = pp.tile([128, NTILE], F32, tag="pz",
                                     name=f"pz{j}_{img}_{rt}")
                        for widx, (k, half) in enumerate(taps):
                            dy, dx = divmod(k, KW)
                            lhsT = w_v[:, k, j * 128 : (j + 1) * 128]
                            xv = xt[(half, img)][:].rearrange(
                                "p (r c) -> p r c", r=HP
                            )
                            rhs = xv[:, rt * RTR + dy : rt * RTR + dy + RTR,
                                     dx : dx + W]
                            nc.tensor.matmul(
                                pt[:], lhsT, rhs,
                                start=(widx == 0), stop=(widx == len(taps) - 1),
                            )
                        col = img * RT + rt
                        zs = z[j][:, img * PIX + rt * NTILE
                                  : img * PIX + (rt + 1) * NTILE]
                        nc.vector.tensor_scalar(
                            out=zs, in0=pt[:], scalar1=0.0, scalar2=None,
                            op0=mybir.AluOpType.add, op1=mybir.AluOpType.add,
                            accum_out=ssum[:, j * npart + col
                                           : j * npart + col + 1],
                        )
                        sqt = sqp.tile([128, NTILE], F32, tag="sqt")
                        nc.scalar.activation(
                            out=sqt[:], in_=pt[:],
                            func=mybir.ActivationFunctionType.Square,
                            accum_out=ssq[:, j * npart + col
                                          : j * npart + col + 1],
                        )

                # ---- chunk-j stats: [128,2] = (sum, sumsq) ----
                cc_sb = sp.tile([128, 2], F32, tag=f"ccsb{j}", name=f"ccsb{j}")
                nc.vector.reduce_sum(
                    out=cc_sb[:, 0:1], in_=ssum[:, j * npart : (j + 1) * npart],
                    axis=mybir.AxisListType.X,
                )
                nc.vector.reduce_sum(
                    out=cc_sb[:, 1:2], in_=ssq[:, j * npart : (j + 1) * npart],
                    axis=mybir.AxisListType.X,
                )
                st = sp.tile([128, 2], F32, tag=f"st{j}", name=f"st{j}")
                if collective and n_cores > 1:
                    cc_in = dp.tile([128, 2], F32, tag=f"ccin{j}",
                                    name=f"ccin{j}")
                    cc_out = dp.tile([128, 2], F32, tag=f"ccout{j}",
                                     name=f"ccout{j}")
                    nc.sync.dma_start(cc_in[:], cc_sb[:])
                    nc.gpsimd.collective_compute(
                        "AllReduce", mybir.AluOpType.add,
                        replica_groups=[list(range(n_cores))],
                        ins=[cc_in.opt()], outs=[cc_out.opt()],
                    )
                    nc.sync.dma_start(st[:], cc_out[:])
                else:
                    nc.vector.tensor_copy(st[:], cc_sb[:])

                # ---- A, B for chunk j:  out = sign(z*A + B) ----
                al, ga, be = (v[:, j : j + 1] for v in (alpha, gamma, beta))
                mu = sp.tile([128, 1], F32, tag=f"mu{j}", name=f"mu{j}")
                var = sp.tile([128, 1], F32, tag=f"var{j}", name=f"var{j}")
                A = sp.tile([128, 1], F32, tag=f"A{j}", name=f"A{j}")
                B = sp.tile([128, 1], F32, tag=f"B{j}", name=f"B{j}")
                tmp = sp.tile([128, 1], F32, tag=f"tmp{j}", name=f"tmp{j}")

                nc.scalar.mul(mu[:], st[:, 0:1], inv_n)          # mu = s/n
                nc.scalar.mul(var[:], st[:, 1:2], inv_n)         # E[z^2]
                nc.vector.tensor_tensor(out=tmp[:], in0=mu[:], in1=mu[:],
                                        op=mybir.AluOpType.mult)
                nc.vector.tensor_tensor(out=var[:], in0=var[:], in1=tmp[:],
                                        op=mybir.AluOpType.subtract)
                nc.vector.tensor_tensor(out=tmp[:], in0=al, in1=al,
                                        op=mybir.AluOpType.mult)
                nc.vector.tensor_tensor(out=var[:], in0=var[:], in1=tmp[:],
                                        op=mybir.AluOpType.mult)
                nc.vector.tensor_scalar(out=var[:], in0=var[:],
                                        scalar1=float(BN_EPS), scalar2=None,
                                        op0=mybir.AluOpType.add)
                nc.scalar.sqrt(var[:], var[:])
                nc.vector.reciprocal(var[:], var[:])     # rsqrt(a^2 var + eps)
                nc.vector.tensor_tensor(out=tmp[:], in0=al, in1=ga,
                                        op=mybir.AluOpType.mult)
                nc.vector.tensor_tensor(out=A[:], in0=tmp[:], in1=var[:],
                                        op=mybir.AluOpType.mult)
                nc.vector.tensor_tensor(out=tmp[:], in0=mu[:], in1=A[:],
                                        op=mybir.AluOpType.mult)
                nc.vector.tensor_tensor(out=B[:], in0=be, in1=tmp[:],
                                        op=mybir.AluOpType.subtract)

                # ---- sign(z*A + B) -> bf16 staging -> DRAM ----
                for img in range(IMGS):
                    ostg = sqp.tile([128, PIX], BF16, tag="ostg",
                                    name=f"ostg{j}_{img}")
                    nc.scalar.activation(
                        out=ostg[:], in_=z[j][:, img * PIX : (img + 1) * PIX],
                        func=mybir.ActivationFunctionType.Sign,
                        bias=B[:, 0:1], scale=A[:, 0:1],
                    )
                    nc.sync.dma_start(out_d[img, j], ostg[:])

    _split_multi_waits(nc)
    return nc


def _prep_inputs(x, weight, gamma, beta):
    """Host-side prep: alpha/sign folding, padding, bf16 hi/lo split."""
    x = np.ascontiguousarray(x, dtype=np.float32)
    weight = np.ascontiguousarray(weight, dtype=np.float32)

    alpha = np.abs(weight).mean(axis=(1, 2, 3)).astype(np.float32)      # [256]
    sgn = np.where(weight >= 0, np.float32(1), np.float32(-1))          # [256,128,3,3]
    # ws[cin, k*256 + o] = sgn[o, cin, dy, dx],  k = dy*3+dx
    ws = np.ascontiguousarray(
        sgn.transpose(1, 2, 3, 0).reshape(CIN, KH * KW * COUT)
    ).astype(ml_dtypes.bfloat16)

    # abg[p, j] layout: [alpha(2) | gamma(2) | beta(2)], channel o = j*128+p
    def chunked(v):
        return np.ascontiguousarray(v.reshape(NCHUNK, 128).T)  # [128, 2]
    abg = np.concatenate(
        [chunked(alpha), chunked(np.asarray(gamma, np.float32)),
         chunked(np.asarray(beta, np.float32))], axis=1
    ).astype(np.float32)                                                # [128, 6]

    xpad = np.zeros((N_FULL, CIN, HP, WP), np.float32)
    xpad[:, :, 1 : H + 1, 1 : W + 1] = x
    xh = xpad.astype(ml_dtypes.bfloat16)
    xl = (xpad - xh.astype(np.float32)).astype(ml_dtypes.bfloat16)
    xh = xh.reshape(N_FULL, CIN, PADPIX)
    xl = xl.reshape(N_FULL, CIN, PADPIX)

    in_maps = []
    for c in range(N_CORES):
        sl = slice(c * IMGS, (c + 1) * IMGS)
        in_maps.append({
            "xh": np.ascontiguousarray(xh[sl]),
            "xl": np.ascontiguousarray(xl[sl]),
            "ws": ws,
            "abg": abg,
        })
    return in_maps


def kernel(x, weight, gamma, beta):
    in_maps = _prep_inputs(x, weight, gamma, beta)
    nc = build_bass()
    res = run_bass_kernel_spmd(nc, in_maps, core_ids=list(range(N_CORES)))
    out = np.empty((N_FULL, COUT, H, W), np.float32)
    for c in range(N_CORES):
        o = res.results[c]["out"]          # [IMGS, 2, 128, 3136] bf16 (+-1)
        o = o.astype(np.float32).reshape(IMGS, COUT, H, W)
        out[c * IMGS : (c + 1) * IMGS] = o
    return out



# revision 3
# speedup vs baseline: 1.9292x; 1.9292x over previous
"""Trainium2 Bass kernel for ConvBnSign (binarized 3x3 conv + sync-BN + sign).

Math: y = conv2d(x, sign(w) * alpha)  with alpha = mean|w| per out-channel,
then train-mode BatchNorm over (N,H,W), then hard_sign.

Since alpha_o > 0 is a per-channel scale, fold it into the BN affine:
  z = conv2d(x, sign(w))          (exact +-1 weights)
  out = sign(z*A + B),  A = alpha*gamma*rsqrt(alpha^2 var_z + eps),
                        B = beta - mu_z*A

Precision: x is split on host into three fp8(e4m3) planes
  hi = Q8(x); mid = Q8((x-hi)*2^4); lo = Q8((x-hi-mid/2^4)*2^6)
with weights sign(w)*(1, 2^-4, 2^-6) per pass, so the recombined conv input
carries ~2^-12 relative error (measured: ~120 sign flips / 25.7M outputs).

PE: fp8 DoubleRow matmuls contract TWO 128x128 tap-blocks per instruction at
0.5 cyc/row -> 27 tap-passes pack into 14 DoubleRow matmuls per output tile
(vs 18 bf16 matmuls in the bf16 hi/lo scheme).  Moving free dim is one
contiguous span of 8 padded rows (464 = 8*58); the 2 garbage columns per row
are skipped when draining PSUM.

Sharding: data-parallel, 4 images per core across 8 cores; BN stats are
per-channel partial sums [128,2] fp32 all-reduced across cores.
"""

import numpy as np
import ml_dtypes

import concourse.bass as bass
import concourse.mybir as mybir
import concourse.tile as tile
from concourse.vector_clock import ScopedClock
from concourse.bass_utils import run_bass_kernel_spmd

# ---- problem constants (hardcoded per contract) ----
N_CORES = 8
N_FULL = 32           # batch
CIN = 128             # input channels
COUT = 256            # output channels
H = W = 56
KH = KW = 3
BN_EPS = 1e-5

IMGS = N_FULL // N_CORES          # 4 images per core
WP = W + 2                        # 58 padded width
HP = H + 2
PADPIX = HP * WP                  # 3364
PSTRIDE = 3368                    # allocated plane stride (tail zeros)
XLEN = 3 * PSTRIDE                # hi|mid|lo planes per image
PIX = H * W                       # 3136
NCHUNK = COUT // 128              # 2 chunks of 128 output channels
RTR = 8                           # output rows per tile
RT = H // RTR                     # 7 row tiles per image
NTILE = RTR * WP                  # 464 = matmul moving free dim (8 padded rows)
NVAL = RTR * W                    # 448 valid pixels per tile
NTOT = N_FULL * PIX               # 200704 elements per channel for BN stats
NPAIR = 14                        # DoubleRow matmuls per (chunk, tile)

BF16 = mybir.dt.bfloat16
F32 = mybir.dt.float32
F8 = mybir.dt.float8e4
E4M3 = ml_dtypes.float8_e4m3

SCALES = (1.0, 2.0 ** -4, 2.0 ** -6)   # hi, mid, lo weight scales

# DoubleRow pair table: ((plane_a, tap_a), (plane_b, tap_b) | None).
# Taps k=0..8 -> (dy,dx)=divmod(k,3).  Planes: 0=hi, 1=mid, 2=lo.
# The LAST pair (stop matmul) must not use the overlapping delta=1 rhs AP:
# walrus/HW faults when stop_tensor_calc pairs with an overlapping ifmap.
PAIRS = (
    [((0, k), (1, k)) for k in range(9)]      # hi/mid of same tap
    + [((2, 0), (2, 1)), ((2, 3), (2, 4)),     # lo taps, in-row neighbors
       ((2, 6), (2, 7)), ((2, 8), None),       # odd lo tap, zero-padded half
       ((2, 2), (2, 5))]                       # lo in-column pair -> stop
)


def _tap_off(plane, k, rt):
    dy, dx = divmod(k, 3)
    return plane * PSTRIDE + (rt * RTR + dy) * WP + dx


_MAX_DRAIN_WAITS = 1  # walrus CTRL instructions accept a single sync wait


def _split_multi_waits(nc, max_waits=1):
    """This walrus build rejects instructions with more than one sem wait.
    Hoist excess waits onto same-engine NoOps inserted immediately before the
    offending instruction (the engine blocks at the NoOp instead — identical
    ordering semantics)."""
    ctr = 0
    for bbw in nc.main_func.blocks:
        out = []
        changed = False
        for inst in bbw.instructions:
            si = inst.sync_info
            w = list(si.on_wait or []) if si else []
            if len(w) > max_waits:
                changed = True
                excess = w[: len(w) - max_waits]
                for i in range(0, len(excess), max_waits):
                    nop = mybir.InstNoOp(name=f"WFIX-{ctr}", ins=[], outs=[])
                    ctr += 1
                    nop.engine = inst.engine
                    nop.sync_info = mybir.SyncInfo(
                        on_wait=excess[i : i + max_waits], on_update=[]
                    )
                    out.append(nop)
                inst.sync_info = mybir.SyncInfo(
                    on_wait=w[len(w) - max_waits :],
                    on_update=list(si.on_update or []),
                )
            out.append(inst)
        if changed:
            bbw.instructions = out
    return ctr


class _SplitDrainTileContext(tile.TileContext):
    """TileContext whose final drain splits its sem waits across multiple
    sync-engine instructions (this walrus build caps CTRL waits at 1)."""

    def _drain_and_barrier(self, tick_clock, wait_clock):
        drain_inst = self.nc.sync.drain()
        wait_clock.add_sem_waits(
            drain_inst.ins, ScopedClock({None: tick_clock.global_clock})
        )
        si = drain_inst.ins.sync_info
        w = list(si.on_wait or [])
        if len(w) > _MAX_DRAIN_WAITS:
            drain_inst.ins.sync_info = mybir.SyncInfo(
                on_wait=w[:_MAX_DRAIN_WAITS], on_update=list(si.on_update or [])
            )
            for i in range(_MAX_DRAIN_WAITS, len(w), _MAX_DRAIN_WAITS):
                nop = self.nc.sync.nop(nofuse=True)
                nop.ins.sync_info = mybir.SyncInfo(
                    on_wait=w[i : i + _MAX_DRAIN_WAITS], on_update=[]
                )
        self.nc.all_engine_barrier()
        assert self.sems is not None
        popped = self.nc._tile_sem_poison_stack.pop()
        assert popped is self._sem_poison
        self.nc.clear_and_free_semaphores(list(self.sems.allocated().values()))
        self.nc.all_engine_barrier()


def build_bass(n_cores=N_CORES, collective=True):
    """Build the per-core Bass module (SPMD: same program on every core)."""
    nc = bass.Bass(num_devices=n_cores)

    xq_d = nc.dram_tensor("xq", [IMGS, CIN, XLEN], F8, kind="ExternalInput")
    ws_d = nc.dram_tensor("ws", [CIN, NCHUNK * NPAIR * 2 * 128], F8,
                          kind="ExternalInput")
    abg_d = nc.dram_tensor("abg", [128, 3 * NCHUNK], F32, kind="ExternalInput")
    out_d = nc.dram_tensor("out", [IMGS, NCHUNK, 128, PIX], BF16,
                           kind="ExternalOutput")

    with _SplitDrainTileContext(nc) as tc:
        with (
            tc.tile_pool(name="const", bufs=1) as constp,
            tc.tile_pool(name="xbuf", bufs=1) as xp,
            tc.tile_pool(name="zbuf", bufs=1) as zp,
            tc.tile_pool(name="stats", bufs=1) as sp,
            tc.tile_pool(name="sq", bufs=2) as sqp,
            tc.tile_pool(name="pz", bufs=8, space="PSUM") as pp,
            tc.tile_pool(name="dram", bufs=1, space="DRAM") as dp,
        ):
            # ---- constants ----
            w_sb = constp.tile([128, NCHUNK * NPAIR * 2 * 128], F8, tag="wpk")
            abg_sb = constp.tile([128, 3 * NCHUNK], F32, tag="abg")
            nc.sync.dma_start(w_sb[:], ws_d[:])
            nc.sync.dma_start(abg_sb[:], abg_d[:])

            # ---- x tiles (per image; hi|mid|lo planes concatenated) ----
            xt = []
            for img in range(IMGS):
                t = xp.tile([128, XLEN], F8, tag=f"x{img}", name=f"x{img}")
                nc.sync.dma_start(t[:], xq_d[img])
                xt.append(t)

            # ---- z buffers + stats ----
            z = [zp.tile([128, IMGS * PIX], F32, tag=f"z{j}", name=f"z{j}")
                 for j in range(NCHUNK)]
            ssum = sp.tile([128, 64], F32, tag="ssum")
            ssq = sp.tile([128, 64], F32, tag="ssq")

            alpha = abg_sb[:, 0:NCHUNK]
            gamma = abg_sb[:, NCHUNK : 2 * NCHUNK]
            beta = abg_sb[:, 2 * NCHUNK : 3 * NCHUNK]
            inv_n = 1.0 / NTOT
            npart = IMGS * RT

            # Per chunk: conv -> stats AllReduce -> sign+store. Chunk 0's
            # collective + BN tail overlaps chunk 1's conv on PE.
            for j in range(NCHUNK):
                for img in range(IMGS):
                    xa = xt[img][:]
                    part_dim = list(xa.ap[0])
                    for rt in range(RT):
                        pt = pp.tile([128, NTILE], F32, tag="pz",
                                     name=f"pz{j}_{img}_{rt}")
                        for q, (ta, tb) in enumerate(PAIRS):
                            off_a = _tap_off(ta[0], ta[1], rt)
                            if tb is None:
                                delta = 1          # zero weights; any finite data
                            else:
                                delta = _tap_off(tb[0], tb[1], rt) - off_a
                            rhs = bass.AP(
                                xa.tensor, xa.offset + off_a,
                                [part_dim, [delta, 2], [1, NTILE]],
                            )
                            woff = (j * NPAIR + q) * 256
                            lhsT = w_sb[:, woff : woff + 256].rearrange(
                                "p (two m) -> p two m", two=2
                            )
                            nc.tensor.matmul(
                                pt[:], lhsT, rhs,
                                start=(q == 0), stop=(q == NPAIR - 1),
                                perf_mode=mybir.MatmulPerfMode.DoubleRow,
                            )
                        # drain valid pixels ([8,56] of the [8,58] span)
                        ptv = pt[:].rearrange("p (r c) -> p r c", r=RTR)[:, :, 0:W]
                        col = img * RT + rt
                        zs = z[j][:, img * PIX + rt * NVAL
                                  : img * PIX + (rt + 1) * NVAL].rearrange(
                            "p (r c) -> p r c", r=RTR)
                        nc.vector.tensor_scalar(
                            out=zs, in0=ptv, scalar1=0.0, scalar2=None,
                            op0=mybir.AluOpType.add, op1=mybir.AluOpType.add,
                            accum_out=ssum[:, j * npart + col
                                           : j * npart + col + 1],
                        )
                        sqt = sqp.tile([128, NVAL], F32, tag="sqt")
                        sqv = sqt[:].rearrange("p (r c) -> p r c", r=RTR)
                        nc.scalar.activation(
                            out=sqv, in_=ptv,
                            func=mybir.ActivationFunctionType.Square,
                            accum_out=ssq[:, j * npart + col
                                          : j * npart + col + 1],
                        )

                # ---- chunk-j stats: [128,2] = (sum, sumsq) ----
                cc_sb = sp.tile([128, 2], F32, tag=f"ccsb{j}", name=f"ccsb{j}")
                nc.vector.reduce_sum(
                    out=cc_sb[:, 0:1], in_=ssum[:, j * npart : (j + 1) * npart],
                    axis=mybir.AxisListType.X,
                )
                nc.vector.reduce_sum(
                    out=cc_sb[:, 1:2], in_=ssq[:, j * npart : (j + 1) * npart],
                    axis=mybir.AxisListType.X,
                )
                st = sp.tile([128, 2], F32, tag=f"st{j}", name=f"st{j}")
                if collective and n_cores > 1:
                    cc_in = dp.tile([128, 2], F32, tag=f"ccin{j}",
                                    name=f"ccin{j}")
                    cc_out = dp.tile([128, 2], F32, tag=f"ccout{j}",
                                     name=f"ccout{j}")
                    nc.sync.dma_start(cc_in[:], cc_sb[:])
                    nc.gpsimd.collective_compute(
                        "AllReduce", mybir.AluOpType.add,
                        replica_groups=[list(range(n_cores))],
                        ins=[cc_in.opt()], outs=[cc_out.opt()],
                    )
                    nc.sync.dma_start(st[:], cc_out[:])
                else:
                    nc.vector.tensor_copy(st[:], cc_sb[:])

                # ---- A, B for chunk j:  out = sign(z*A + B) ----
                al, ga, be = (v[:, j : j + 1] for v in (alpha, gamma, beta))
                mu = sp.tile([128, 1], F32, tag=f"mu{j}", name=f"mu{j}")
                var = sp.tile([128, 1], F32, tag=f"var{j}", name=f"var{j}")
                A = sp.tile([128, 1], F32, tag=f"A{j}", name=f"A{j}")
                B = sp.tile([128, 1], F32, tag=f"B{j}", name=f"B{j}")
                tmp = sp.tile([128, 1], F32, tag=f"tmp{j}", name=f"tmp{j}")

                nc.scalar.mul(mu[:], st[:, 0:1], inv_n)          # mu = s/n
                nc.scalar.mul(var[:], st[:, 1:2], inv_n)         # E[z^2]
                nc.vector.tensor_tensor(out=tmp[:], in0=mu[:], in1=mu[:],
                                        op=mybir.AluOpType.mult)
                nc.vector.tensor_tensor(out=var[:], in0=var[:], in1=tmp[:],
                                        op=mybir.AluOpType.subtract)
                nc.vector.tensor_tensor(out=tmp[:], in0=al, in1=al,
                                        op=mybir.AluOpType.mult)
                nc.vector.tensor_tensor(out=var[:], in0=var[:], in1=tmp[:],
                                        op=mybir.AluOpType.mult)
                nc.vector.tensor_scalar(out=var[:], in0=var[:],
                                        scalar1=float(BN_EPS), scalar2=None,
                                        op0=mybir.AluOpType.add)
                nc.scalar.sqrt(var[:], var[:])
                nc.vector.reciprocal(var[:], var[:])     # rsqrt(a^2 var + eps)
                nc.vector.tensor_tensor(out=tmp[:], in0=al, in1=ga,
                                        op=mybir.AluOpType.mult)
                nc.vector.tensor_tensor(out=A[:], in0=tmp[:], in1=var[:],
                                        op=mybir.AluOpType.mult)
                nc.vector.tensor_tensor(out=tmp[:], in0=mu[:], in1=A[:],
                                        op=mybir.AluOpType.mult)
                nc.vector.tensor_tensor(out=B[:], in0=be, in1=tmp[:],
                                        op=mybir.AluOpType.subtract)

                # ---- sign(z*A + B) -> bf16 staging -> DRAM ----
                for img in range(IMGS):
                    ostg = sqp.tile([128, PIX], BF16, tag="ostg",
                                    name=f"ostg{j}_{img}")
                    nc.scalar.activation(
                        out=ostg[:], in_=z[j][:, img * PIX : (img + 1) * PIX],
                        func=mybir.ActivationFunctionType.Sign,
                        bias=B[:, 0:1], scale=A[:, 0:1],
                    )
                    nc.sync.dma_start(out_d[img, j], ostg[:])

    _split_multi_waits(nc)
    return nc


def _prep_inputs(x, weight, gamma, beta):
    """Host-side prep: alpha/sign folding, padding, fp8 hi/mid/lo split."""
    x = np.ascontiguousarray(x, dtype=np.float32)
    weight = np.ascontiguousarray(weight, dtype=np.float32)

    alpha = np.abs(weight).mean(axis=(1, 2, 3)).astype(np.float32)      # [256]
    sgn = np.where(weight >= 0, np.float32(1), np.float32(-1))          # [256,128,3,3]

    # DoubleRow-packed weights: wpk[cin, chunk, pair, half, m] = sgn * scale
    wpk = np.zeros((CIN, NCHUNK, NPAIR, 2, 128), np.float32)
    for j in range(NCHUNK):
        for q, (ta, tb) in enumerate(PAIRS):
            for h, t in ((0, ta), (1, tb)):
                if t is None:
                    continue
                plane, k = t
                dy, dx = divmod(k, 3)
                wpk[:, j, q, h, :] = (
                    sgn[j * 128 : (j + 1) * 128, :, dy, dx].T * SCALES[plane]
                )
    ws = np.ascontiguousarray(
        wpk.reshape(CIN, NCHUNK * NPAIR * 2 * 128)
    ).astype(E4M3)

    # abg[p, j] layout: [alpha(2) | gamma(2) | beta(2)], channel o = j*128+p
    def chunked(v):
        return np.ascontiguousarray(v.reshape(NCHUNK, 128).T)  # [128, 2]
    abg = np.concatenate(
        [chunked(alpha), chunked(np.asarray(gamma, np.float32)),
         chunked(np.asarray(beta, np.float32))], axis=1
    ).astype(np.float32)                                                # [128, 6]

    # fp8 hi/mid/lo split of the padded input
    xpad = np.zeros((N_FULL, CIN, HP * WP), np.float32)
    xpad.reshape(N_FULL, CIN, HP, WP)[:, :, 1 : H + 1, 1 : W + 1] = x
    hi_q = xpad.astype(E4M3)
    r1 = xpad - hi_q.astype(np.float32)
    mid_q = (r1 * 16.0).astype(E4M3)
    r2 = r1 - mid_q.astype(np.float32) * (1.0 / 16.0)
    lo_q = (r2 * 64.0).astype(E4M3)

    xq = np.zeros((N_FULL, CIN, 3, PSTRIDE), E4M3)
    xq[:, :, 0, :PADPIX] = hi_q
    xq[:, :, 1, :PADPIX] = mid_q
    xq[:, :, 2, :PADPIX] = lo_q
    xq = xq.reshape(N_FULL, CIN, XLEN)

    in_maps = []
    for c in range(N_CORES):
        sl = slice(c * IMGS, (c + 1) * IMGS)
        in_maps.append({
            "xq": np.ascontiguousarray(xq[sl]),
            "ws": ws,
            "abg": abg,
        })
    return in_maps


def kernel(x, weight, gamma, beta):
    in_maps = _prep_inputs(x, weight, gamma, beta)
    nc = build_bass()
    res = run_bass_kernel_spmd(nc, in_maps, core_ids=list(range(N_CORES)))
    out = np.empty((N_FULL, COUT, H, W), np.float32)
    for c in range(N_CORES):
        o = res.results[c]["out"]          # [IMGS, 2, 128, 3136] bf16 (+-1)
        o = o.astype(np.float32).reshape(IMGS, COUT, H, W)
        out[c * IMGS : (c + 1) * IMGS] = o
    return out


# revision 23
# speedup vs baseline: 2.3277x; 1.2066x over previous
"""Trainium2 Bass kernel for ConvBnSign (binarized 3x3 conv + sync-BN + sign).

Math: y = conv2d(x, sign(w) * alpha)  with alpha = mean|w| per out-channel,
then train-mode BatchNorm over (N,H,W), then hard_sign.

Since alpha_o > 0 is a per-channel scale, fold it into the BN affine:
  z = conv2d(x, sign(w))          (exact +-1 weights)
  out = sign(z*A + B),  A = alpha*gamma*rsqrt(alpha^2 var_z + eps),
                        B = beta - mu_z*A

Precision: x is split on host into three fp8(e4m3) planes
  hi = Q8(x); mid = Q8((x-hi)*2^4); lo = Q8((x-hi-mid/2^4)*2^6)
with weights sign(w)*(1, 2^-4, 2^-6) per pass, so the recombined conv input
carries ~2^-12 relative error (measured: ~120 sign flips / 25.7M outputs).

PE: fp8 DoubleRow matmuls contract TWO 128x128 tap-blocks per instruction at
0.5 cyc/row -> 27 tap-passes pack into 14 DoubleRow matmuls per output tile
(vs 18 bf16 matmuls in the bf16 hi/lo scheme).  Moving free dim is one
contiguous span of 8 padded rows (464 = 8*58); the 2 garbage columns per row
are skipped when draining PSUM.

Sharding: data-parallel, 4 images per core across 8 cores; BN stats are
per-channel partial sums [128,2] fp32 all-reduced across cores.
"""

import numpy as np
import ml_dtypes

import concourse.bass as bass
import concourse.mybir as mybir
import concourse.tile as tile
from concourse.vector_clock import ScopedClock
from concourse.bass_utils import run_bass_kernel_spmd

# ---- problem constants (hardcoded per contract) ----
N_CORES = 8
N_FULL = 32           # batch
CIN = 128             # input channels
COUT = 256            # output channels
H = W = 56
KH = KW = 3
BN_EPS = 1e-5

IMGS = N_FULL // N_CORES          # 4 images per core
WP = W + 2                        # 58 padded width
HP = H + 2
PADPIX = HP * WP                  # 3364
PSTRIDE = 3368                    # allocated plane stride (tail zeros)
XLEN = 3 * PSTRIDE                # hi|mid|lo planes per image
PIX = H * W                       # 3136
NCHUNK = COUT // 128              # 2 chunks of 128 output channels
RTR = 8                           # output rows per tile
RT = H // RTR                     # 7 row tiles per image
NTILE = RTR * WP                  # 464 = matmul moving free dim (8 padded rows)
NVAL = RTR * W                    # 448 valid pixels per tile
NTOT = N_FULL * PIX               # 200704 elements per channel for BN stats
NPAIR = 14                        # DoubleRow matmuls per (chunk, tile)

BF16 = mybir.dt.bfloat16
F32 = mybir.dt.float32
F8 = mybir.dt.float8e4
E4M3 = ml_dtypes.float8_e4m3

SCALES = (1.0, 2.0 ** -4, 2.0 ** -6)   # hi, mid, lo weight scales

# DoubleRow pair table: ((plane_a, tap_a), (plane_b, tap_b) | None).
# Taps k=0..8 -> (dy,dx)=divmod(k,3).  Planes: 0=hi, 1=mid, 2=lo.
# The LAST pair (stop matmul) must not use the overlapping delta=1 rhs AP:
# walrus/HW faults when stop_tensor_calc pairs with an overlapping ifmap.
PAIRS = (
    [((0, k), (1, k)) for k in range(9)]      # hi/mid of same tap
    + [((2, 0), (2, 1)), ((2, 3), (2, 4)),     # lo taps, in-row neighbors
       ((2, 6), (2, 7)), ((2, 8), None),       # odd lo tap, zero-padded half
       ((2, 2), (2, 5))]                       # lo in-column pair -> stop
)


def _tap_off(plane, k, rt):
    dy, dx = divmod(k, 3)
    return plane * PSTRIDE + (rt * RTR + dy) * WP + dx


_MAX_DRAIN_WAITS = 1  # walrus CTRL instructions accept a single sync wait


def _split_multi_waits(nc, max_waits=1):
    """This walrus build rejects instructions with more than one sem wait.
    Hoist excess waits onto same-engine NoOps inserted immediately before the
    offending instruction (the engine blocks at the NoOp instead — identical
    ordering semantics)."""
    ctr = 0
    for bbw in nc.main_func.blocks:
        out = []
        changed = False
        for inst in bbw.instructions:
            si = inst.sync_info
            w = list(si.on_wait or []) if si else []
            if len(w) > max_waits:
                changed = True
                excess = w[: len(w) - max_waits]
                for i in range(0, len(excess), max_waits):
                    nop = mybir.InstNoOp(name=f"WFIX-{ctr}", ins=[], outs=[])
                    ctr += 1
                    nop.engine = inst.engine
                    nop.sync_info = mybir.SyncInfo(
                        on_wait=excess[i : i + max_waits], on_update=[]
                    )
                    out.append(nop)
                inst.sync_info = mybir.SyncInfo(
                    on_wait=w[len(w) - max_waits :],
                    on_update=list(si.on_update or []),
                )
            out.append(inst)
        if changed:
            bbw.instructions = out
    return ctr


class _SplitDrainTileContext(tile.TileContext):
    """TileContext whose final drain splits its sem waits across multiple
    sync-engine instructions (this walrus build caps CTRL waits at 1)."""

    def _drain_and_barrier(self, tick_clock, wait_clock):
        drain_inst = self.nc.sync.drain()
        wait_clock.add_sem_waits(
            drain_inst.ins, ScopedClock({None: tick_clock.global_clock})
        )
        si = drain_inst.ins.sync_info
        w = list(si.on_wait or [])
        if len(w) > _MAX_DRAIN_WAITS:
            drain_inst.ins.sync_info = mybir.SyncInfo(
                on_wait=w[:_MAX_DRAIN_WAITS], on_update=list(si.on_update or [])
            )
            for i in range(_MAX_DRAIN_WAITS, len(w), _MAX_DRAIN_WAITS):
                nop = self.nc.sync.nop(nofuse=True)
                nop.ins.sync_info = mybir.SyncInfo(
                    on_wait=w[i : i + _MAX_DRAIN_WAITS], on_update=[]
                )
        self.nc.all_engine_barrier()
        assert self.sems is not None
        popped = self.nc._tile_sem_poison_stack.pop()
        assert popped is self._sem_poison
        self.nc.clear_and_free_semaphores(list(self.sems.allocated().values()))
        self.nc.all_engine_barrier()


def build_bass(n_cores=N_CORES, collective=True, fast_bn=True):
    """Build the per-core Bass module (SPMD: same program on every core).

    fast_bn: beta==0 specialization — sign(A*(z-mu)+0) == sign(gamma) *
    sign(z-mu) exactly (A = alpha*gamma*rsqrt(..) and alpha,rsqrt > 0), so
    the variance/sumsq pipeline is skipped entirely and only the per-channel
    sum is all-reduced.
    """
    nc = bass.Bass(num_devices=n_cores)

    xq_d = nc.dram_tensor("xq", [IMGS, CIN, XLEN], F8, kind="ExternalInput")
    ws_d = nc.dram_tensor("ws", [CIN, NCHUNK * NPAIR * 2 * 128], F8,
                          kind="ExternalInput")
    abg_d = nc.dram_tensor("abg", [128, 3 * NCHUNK], F32, kind="ExternalInput")
    out_d = nc.dram_tensor("out", [IMGS, NCHUNK, 128, PIX], F8,
                           kind="ExternalOutput")
    # tail signs offloaded to DVE (bf16 bit trick): img0 full + img1 1st half
    out2_d = nc.dram_tensor("out2", [2, 128, PIX], BF16, kind="ExternalOutput")

    with _SplitDrainTileContext(nc) as tc:
        with (
            tc.tile_pool(name="const", bufs=1) as constp,
            tc.tile_pool(name="xbuf", bufs=1) as xp,
            tc.tile_pool(name="zbuf", bufs=1) as zp,
            tc.tile_pool(name="stats", bufs=1) as sp,
            tc.tile_pool(name="sq", bufs=2) as sqp,
            tc.tile_pool(name="ostg", bufs=4) as op_,
            tc.tile_pool(name="pz", bufs=8, space="PSUM") as pp,
            tc.tile_pool(name="dram", bufs=1, space="DRAM") as dp,
        ):
            # ---- constants (weights split per chunk so chunk 0 loads fast) ----
            w_sb = [constp.tile([128, NPAIR * 2 * 128], F8, tag=f"wpk{j}",
                                name=f"wpk{j}") for j in range(NCHUNK)]
            abg_sb = constp.tile([128, 3 * NCHUNK], F32, tag="abg")
            wv = ws_d[:].rearrange("p (j r) -> p j r", j=NCHUNK)
            nc.sync.dma_start(w_sb[0][:], wv[:, 0])

            # ---- x tiles; image 0 loads per-plane so PE starts early ----
            xt = []
            for img in range(IMGS):
                t = xp.tile([128, XLEN], F8, tag=f"x{img}", name=f"x{img}")
                xt.append(t)
            # hi+mid planes in one transfer (first matmuls), lo separately
            nc.sync.dma_start(xt[0][:, : 2 * PSTRIDE],
                              xq_d[0][:, : 2 * PSTRIDE])
            nc.sync.dma_start(xt[0][:, 2 * PSTRIDE :],
                              xq_d[0][:, 2 * PSTRIDE :])
            nc.sync.dma_start(w_sb[1][:], wv[:, 1])
            for img in range(1, IMGS):
                nc.sync.dma_start(xt[img][:], xq_d[img])
            nc.sync.dma_start(abg_sb[:], abg_d[:])

            # ---- z buffers + stats ----
            z = [zp.tile([128, IMGS * PIX], F32, tag=f"z{j}", name=f"z{j}")
                 for j in range(NCHUNK)]
            ssum = sp.tile([128, 64], F32, tag="ssum")
            ssq = None if fast_bn else sp.tile([128, 64], F32, tag="ssq")

            # host-precomputed columns (see _prep_inputs):
            #  fast_bn: ag=sign(gamma), na2=-sign(gamma), be unused
            #  general: ag=alpha*gamma, na2=-alpha^2,     be=beta
            ag = abg_sb[:, 0:NCHUNK]
            na2 = abg_sb[:, NCHUNK : 2 * NCHUNK]
            be = abg_sb[:, 2 * NCHUNK : 3 * NCHUNK]
            inv_n = 1.0 / NTOT
            npart = IMGS * RT

            AB = {}

            def emit_sign(j, img, lo, hi):
                """sign(z*A+B) for pixels [lo,hi) of (chunk j, img) -> DRAM."""
                A, B = AB[j]
                ostg = op_.tile([128, hi - lo], F8, tag="ostg",
                                name=f"ostg{j}_{img}_{lo}")
                nc.scalar.activation(
                    out=ostg[:], in_=z[j][:, img * PIX + lo : img * PIX + hi],
                    func=mybir.ActivationFunctionType.Sign,
                    bias=B[:, 0:1], scale=A[:, 0:1],
                )
                nc.sync.dma_start(out_d[img, j][:, lo:hi], ostg[:])

            # chunk-0 sign work, split into quarter-images, interleaved into
            # chunk-1's conv loop so the ACT engine never bursts
            c0_pieces = [(img, lo, lo + PIX // 4)
                         for img in range(IMGS)
                         for lo in range(0, PIX, PIX // 4)]

            # Per chunk: conv -> stats AllReduce -> sign+store. Chunk 0's
            # collective + BN tail overlaps chunk 1's conv on PE.
            for j in range(NCHUNK):
                unit = 0
                for img in range(IMGS):
                    xa = xt[img][:]
                    part_dim = list(xa.ap[0])
                    for rt in range(RT):
                        pt = pp.tile([128, NVAL], F32, tag="pz",
                                     name=f"pz{j}_{img}_{rt}")
                        for q, (ta, tb) in enumerate(PAIRS):
                            off_a = _tap_off(ta[0], ta[1], rt)
                            if tb is None:
                                delta = 1          # zero weights; any finite data
                            else:
                                delta = _tap_off(tb[0], tb[1], rt) - off_a
                            # moving free = [pair, row, col]; only the 448
                            # valid pixels are computed (rows at stride WP)
                            rhs = bass.AP(
                                xa.tensor, xa.offset + off_a,
                                [part_dim, [delta, 2], [WP, RTR], [1, W]],
                            )
                            woff = q * 256
                            lhsT = w_sb[j][:, woff : woff + 256].rearrange(
                                "p (two m) -> p two m", two=2
                            )
                            nc.tensor.matmul(
                                pt[:], lhsT, rhs,
                                start=(q == 0), stop=(q == NPAIR - 1),
                                perf_mode=mybir.MatmulPerfMode.DoubleRow,
                            )
                        col = img * RT + rt
                        zs = z[j][:, img * PIX + rt * NVAL
                                  : img * PIX + (rt + 1) * NVAL]
                        nc.vector.tensor_scalar(
                            out=zs, in0=pt[:], scalar1=0.0, scalar2=None,
                            op0=mybir.AluOpType.add, op1=mybir.AluOpType.add,
                            accum_out=ssum[:, j * npart + col
                                           : j * npart + col + 1],
                        )
                        if not fast_bn:
                            sqt = sqp.tile([128, NVAL], F32, tag="sqt")
                            nc.scalar.activation(
                                out=sqt[:], in_=pt[:],
                                func=mybir.ActivationFunctionType.Square,
                                accum_out=ssq[:, j * npart + col
                                              : j * npart + col + 1],
                            )
                        if j == 1 and unit < len(c0_pieces):
                            emit_sign(0, *c0_pieces[unit])
                        unit += 1

                # ---- chunk-j stats: [128,SW] = (sum[, sumsq]) ----
                SW = 1 if fast_bn else 2
                cc_sb = sp.tile([128, SW], F32, tag=f"ccsb{j}", name=f"ccsb{j}")
                nc.vector.reduce_sum(
                    out=cc_sb[:, 0:1], in_=ssum[:, j * npart : (j + 1) * npart],
                    axis=mybir.AxisListType.X,
                )
                if not fast_bn:
                    nc.vector.reduce_sum(
                        out=cc_sb[:, 1:2],
                        in_=ssq[:, j * npart : (j + 1) * npart],
                        axis=mybir.AxisListType.X,
                    )
                st = sp.tile([128, SW], F32, tag=f"st{j}", name=f"st{j}")
                if collective and n_cores > 1:
                    cc_in = dp.tile([128, SW], F32, tag=f"ccin{j}",
                                    name=f"ccin{j}")
                    cc_out = dp.tile([128, SW], F32, tag=f"ccout{j}",
                                     name=f"ccout{j}")
                    nc.sync.dma_start(cc_in[:], cc_sb[:])
                    nc.gpsimd.collective_compute(
                        "AllReduce", mybir.AluOpType.add,
                        replica_groups=[list(range(n_cores))],
                        ins=[cc_in.opt()], outs=[cc_out.opt()],
                    )
                    nc.sync.dma_start(st[:], cc_out[:])
                else:
                    nc.vector.tensor_copy(st[:], cc_sb[:])

                B = sp.tile([128, 1], F32, tag=f"B{j}", name=f"B{j}")
                if fast_bn:
                    # beta == 0: sign(A*(z-mu)) == sign(gamma)*sign(z-mu);
                    # abg carries sg=sign(gamma) and nsg=-sign(gamma).
                    # A = sg (host constant);  B = mu*nsg = -mu*sg
                    A = ag[:, j : j + 1]
                    nc.vector.tensor_scalar(
                        out=B[:], in0=st[:, 0:1], scalar1=inv_n,
                        scalar2=na2[:, j : j + 1],
                        op0=mybir.AluOpType.mult, op1=mybir.AluOpType.mult)
                else:
                    A = sp.tile([128, 1], F32, tag=f"A{j}", name=f"A{j}")
                    # ms=(mu,m2); nv=mu^2-m2=-var; v2=nv*(-a2)+eps;
                    # A = ag/sqrt(v2); B = beta - mu*A
                    ms = sp.tile([128, 2], F32, tag=f"ms{j}", name=f"ms{j}")
                    nv = sp.tile([128, 1], F32, tag=f"nv{j}", name=f"nv{j}")
                    tmp = sp.tile([128, 1], F32, tag=f"tmp{j}", name=f"tmp{j}")
                    nc.vector.tensor_scalar(out=ms[:], in0=st[:],
                                            scalar1=inv_n, scalar2=None,
                                            op0=mybir.AluOpType.mult)
                    nc.vector.scalar_tensor_tensor(
                        out=nv[:], in0=ms[:, 0:1], scalar=ms[:, 0:1],
                        in1=ms[:, 1:2], op0=mybir.AluOpType.mult,
                        op1=mybir.AluOpType.subtract)
                    nc.vector.tensor_scalar(
                        out=tmp[:], in0=nv[:], scalar1=na2[:, j : j + 1],
                        scalar2=float(BN_EPS), op0=mybir.AluOpType.mult,
                        op1=mybir.AluOpType.add)
                    nc.scalar.sqrt(tmp[:], tmp[:])
                    nc.vector.reciprocal(tmp[:], tmp[:])  # rsqrt(a^2 var+eps)
                    nc.vector.tensor_scalar(out=A[:], in0=tmp[:],
                                            scalar1=ag[:, j : j + 1],
                                            scalar2=None,
                                            op0=mybir.AluOpType.mult)
                    nc.vector.tensor_tensor(out=tmp[:], in0=ms[:, 0:1],
                                            in1=A[:], op=mybir.AluOpType.mult)
                    nc.vector.tensor_tensor(out=B[:], in0=be[:, j : j + 1],
                                            in1=tmp[:],
                                            op=mybir.AluOpType.subtract)
                AB[j] = (A, B)

                # chunk-1 signs run in the tail, split ACT/DVE.  DVE bit
                # trick: sign(y) == (y & 0x8000) | 0x3f80 on the bf16
                # encoding (rounding y to bf16 preserves its sign exactly).
                if j == NCHUNK - 1:
                    def dve_sign(row, img, lo, hi):
                        n = hi - lo
                        ybf = sqp.tile([128, n], BF16, tag="ybf",
                                       name=f"ybf{row}")
                        nc.vector.tensor_scalar(
                            out=ybf[:],
                            in0=z[j][:, img * PIX + lo : img * PIX + hi],
                            scalar1=A[:, 0:1], scalar2=B[:, 0:1],
                            op0=mybir.AluOpType.mult, op1=mybir.AluOpType.add,
                        )
                        osg2 = op_.tile([128, n], BF16, tag="osg2",
                                        name=f"osg2_{row}")
                        nc.vector.tensor_scalar(
                            out=osg2[:].bitcast(mybir.dt.int16),
                            in0=ybf[:].bitcast(mybir.dt.int16),
                            scalar1=0x8000, scalar2=0x3F80,
                            op0=mybir.AluOpType.bitwise_and,
                            op1=mybir.AluOpType.bitwise_or,
                        )
                        nc.sync.dma_start(out2_d[row][:, lo:hi], osg2[:])

                    dve_sign(0, 0, 0, PIX)             # img0 full on DVE
                    dve_sign(1, 1, 0, PIX // 2)        # img1 1st half on DVE
                    emit_sign(j, 2, 0, PIX)            # ACT
                    emit_sign(j, 3, 0, PIX)            # ACT
                    emit_sign(j, 1, PIX // 2, PIX)     # ACT, small flush last

    _split_multi_waits(nc)
    return nc


def _prep_inputs(x, weight, gamma, beta, fast_bn=True):
    """Host-side prep: alpha/sign folding, padding, fp8 hi/mid/lo split."""
    x = np.ascontiguousarray(x, dtype=np.float32)
    weight = np.ascontiguousarray(weight, dtype=np.float32)

    alpha = np.abs(weight).mean(axis=(1, 2, 3)).astype(np.float32)      # [256]
    sgn = np.where(weight >= 0, np.float32(1), np.float32(-1))          # [256,128,3,3]

    # DoubleRow-packed weights: wpk[cin, chunk, pair, half, m] = sgn * scale
    wpk = np.zeros((CIN, NCHUNK, NPAIR, 2, 128), np.float32)
    for j in range(NCHUNK):
        for q, (ta, tb) in enumerate(PAIRS):
            for h, t in ((0, ta), (1, tb)):
                if t is None:
                    continue
                plane, k = t
                dy, dx = divmod(k, 3)
                wpk[:, j, q, h, :] = (
                    sgn[j * 128 : (j + 1) * 128, :, dy, dx].T * SCALES[plane]
                )
    ws = np.ascontiguousarray(
        wpk.reshape(CIN, NCHUNK * NPAIR * 2 * 128)
    ).astype(E4M3)

    # abg[p, j] columns (channel o = j*128+p):
    #  fast_bn: [sign(gamma) | -sign(gamma) | 0]
    #  general: [alpha*gamma | -alpha^2    | beta]
    def chunked(v):
        return np.ascontiguousarray(v.reshape(NCHUNK, 128).T)  # [128, 2]
    gamma = np.asarray(gamma, np.float32)
    if fast_bn:
        sg = np.where(gamma >= 0, np.float32(1), np.float32(-1))
        cols = [chunked(sg), chunked(-sg), chunked(np.zeros_like(sg))]
    else:
        cols = [chunked(alpha * gamma), chunked(-alpha * alpha),
                chunked(np.asarray(beta, np.float32))]
    abg = np.concatenate(cols, axis=1).astype(np.float32)               # [128, 6]

    # fp8 hi/mid/lo split of the padded input
    xpad = np.zeros((N_FULL, CIN, HP * WP), np.float32)
    xpad.reshape(N_FULL, CIN, HP, WP)[:, :, 1 : H + 1, 1 : W + 1] = x
    hi_q = xpad.astype(E4M3)
    r1 = xpad - hi_q.astype(np.float32)
    mid_q = (r1 * 16.0).astype(E4M3)
    r2 = r1 - mid_q.astype(np.float32) * (1.0 / 16.0)
    lo_q = (r2 * 64.0).astype(E4M3)

    xq = np.zeros((N_FULL, CIN, 3, PSTRIDE), E4M3)
    xq[:, :, 0, :PADPIX] = hi_q
    xq[:, :, 1, :PADPIX] = mid_q
    xq[:, :, 2, :PADPIX] = lo_q
    xq = xq.reshape(N_FULL, CIN, XLEN)

    in_maps = []
    for c in range(N_CORES):
        sl = slice(c * IMGS, (c + 1) * IMGS)
        in_maps.append({
            "xq": np.ascontiguousarray(xq[sl]),
            "ws": ws,
            "abg": abg,
        })
    return in_maps


def kernel(x, weight, gamma, beta):
    fast_bn = bool(np.all(np.asarray(beta) == 0))
    in_maps = _prep_inputs(x, weight, gamma, beta, fast_bn=fast_bn)
    nc = build_bass(fast_bn=fast_bn)
    res = run_bass_kernel_spmd(nc, in_maps, core_ids=list(range(N_CORES)))
    out = np.empty((N_FULL, COUT, H, W), np.float32)
    for c in range(N_CORES):
        o = res.results[c]["out"].astype(np.float32)  # [IMGS,2,128,3136] +-1
        o2 = res.results[c]["out2"].astype(np.float32)  # DVE-signed parts
        o[0, 1] = o2[0]
        o[1, 1, :, : PIX // 2] = o2[1][:, : PIX // 2]
        out[c * IMGS : (c + 1) * IMGS] = o.reshape(IMGS, COUT, H, W)
    return out


# revision 33
# speedup vs baseline: 2.4102x; 1.0355x over previous
"""Trainium2 Bass kernel for ConvBnSign (binarized 3x3 conv + sync-BN + sign).

Math: y = conv2d(x, sign(w) * alpha)  with alpha = mean|w| per out-channel,
then train-mode BatchNorm over (N,H,W), then hard_sign.

Since alpha_o > 0 is a per-channel scale, fold it into the BN affine:
  z = conv2d(x, sign(w))          (exact +-1 weights)
  out = sign(z*A + B),  A = alpha*gamma*rsqrt(alpha^2 var_z + eps),
                        B = beta - mu_z*A

Precision: x is split on host into three fp8(e4m3) planes
  hi = Q8(x); mid = Q8((x-hi)*2^4); lo = Q8((x-hi-mid/2^4)*2^6)
with weights sign(w)*(1, 2^-4, 2^-6) per pass, so the recombined conv input
carries ~2^-12 relative error (measured: ~120 sign flips / 25.7M outputs).

PE: fp8 DoubleRow matmuls contract TWO 128x128 tap-blocks per instruction at
0.5 cyc/row -> 27 tap-passes pack into 14 DoubleRow matmuls per output tile
(vs 18 bf16 matmuls in the bf16 hi/lo scheme).  Moving free dim is one
contiguous span of 8 padded rows (464 = 8*58); the 2 garbage columns per row
are skipped when draining PSUM.

Sharding: data-parallel, 4 images per core across 8 cores; BN stats are
per-channel partial sums [128,2] fp32 all-reduced across cores.
"""

import numpy as np
import ml_dtypes

import concourse.bass as bass
import concourse.mybir as mybir
import concourse.tile as tile
from concourse.vector_clock import ScopedClock
from concourse.bass_utils import run_bass_kernel_spmd

# ---- problem constants (hardcoded per contract) ----
N_CORES = 8
N_FULL = 32           # batch
CIN = 128             # input channels
COUT = 256            # output channels
H = W = 56
KH = KW = 3
BN_EPS = 1e-5

IMGS = N_FULL // N_CORES          # 4 images per core
WP = W + 2                        # 58 padded width
HP = H + 2
PADPIX = HP * WP                  # 3364
ROWSTR = 3 * WP                   # 174: planes interleaved per padded row
PLOFF = WP                        # plane p of row r at r*ROWSTR + p*PLOFF
XLEN = HP * ROWSTR + 12           # 10104 (12 tail zeros for AP margins)
PIX = H * W                       # 3136
NCHUNK = COUT // 128              # 2 chunks of 128 output channels
RTR = 8                           # output rows per tile
RT = H // RTR                     # 7 row tiles per image
NTILE = RTR * WP                  # 464 = matmul moving free dim (8 padded rows)
NVAL = RTR * W                    # 448 valid pixels per tile
NTOT = N_FULL * PIX               # 200704 elements per channel for BN stats
NPAIR = 14                        # DoubleRow matmuls per (chunk, tile)

BF16 = mybir.dt.bfloat16
F32 = mybir.dt.float32
F8 = mybir.dt.float8e4
E4M3 = ml_dtypes.float8_e4m3

SCALES = (1.0, 2.0 ** -4, 2.0 ** -6)   # hi, mid, lo weight scales

# DoubleRow pair table: ((plane_a, tap_a), (plane_b, tap_b) | None).
# Taps k=0..8 -> (dy,dx)=divmod(k,3).  Planes: 0=hi, 1=mid, 2=lo.
# The LAST pair (stop matmul) must not use the overlapping delta=1 rhs AP:
# walrus/HW faults when stop_tensor_calc pairs with an overlapping ifmap.
PAIRS = (
    [((0, k), (1, k)) for k in range(9)]      # hi/mid of same tap
    + [((2, 0), (2, 1)), ((2, 3), (2, 4)),     # lo taps, in-row neighbors
       ((2, 6), (2, 7)), ((2, 8), None),       # odd lo tap, zero-padded half
       ((2, 2), (2, 5))]                       # lo in-column pair -> stop
)


def _tap_off(plane, k, rt):
    dy, dx = divmod(k, 3)
    return (rt * RTR + dy) * ROWSTR + plane * PLOFF + dx


_MAX_DRAIN_WAITS = 1  # walrus CTRL instructions accept a single sync wait


def _split_multi_waits(nc, max_waits=1):
    """This walrus build rejects instructions with more than one sem wait.
    Hoist excess waits onto same-engine NoOps inserted immediately before the
    offending instruction (the engine blocks at the NoOp instead — identical
    ordering semantics)."""
    ctr = 0
    for bbw in nc.main_func.blocks:
        out = []
        changed = False
        for inst in bbw.instructions:
            si = inst.sync_info
            w = list(si.on_wait or []) if si else []
            if len(w) > max_waits:
                changed = True
                excess = w[: len(w) - max_waits]
                for i in range(0, len(excess), max_waits):
                    nop = mybir.InstNoOp(name=f"WFIX-{ctr}", ins=[], outs=[])
                    ctr += 1
                    nop.engine = inst.engine
                    nop.sync_info = mybir.SyncInfo(
                        on_wait=excess[i : i + max_waits], on_update=[]
                    )
                    out.append(nop)
                inst.sync_info = mybir.SyncInfo(
                    on_wait=w[len(w) - max_waits :],
                    on_update=list(si.on_update or []),
                )
            out.append(inst)
        if changed:
            bbw.instructions = out
    return ctr


class _SplitDrainTileContext(tile.TileContext):
    """TileContext whose final drain splits its sem waits across multiple
    sync-engine instructions (this walrus build caps CTRL waits at 1)."""

    def _drain_and_barrier(self, tick_clock, wait_clock):
        drain_inst = self.nc.sync.drain()
        wait_clock.add_sem_waits(
            drain_inst.ins, ScopedClock({None: tick_clock.global_clock})
        )
        si = drain_inst.ins.sync_info
        w = list(si.on_wait or [])
        if len(w) > _MAX_DRAIN_WAITS:
            drain_inst.ins.sync_info = mybir.SyncInfo(
                on_wait=w[:_MAX_DRAIN_WAITS], on_update=list(si.on_update or [])
            )
            for i in range(_MAX_DRAIN_WAITS, len(w), _MAX_DRAIN_WAITS):
                nop = self.nc.sync.nop(nofuse=True)
                nop.ins.sync_info = mybir.SyncInfo(
                    on_wait=w[i : i + _MAX_DRAIN_WAITS], on_update=[]
                )
        self.nc.all_engine_barrier()
        assert self.sems is not None
        popped = self.nc._tile_sem_poison_stack.pop()
        assert popped is self._sem_poison
        self.nc.clear_and_free_semaphores(list(self.sems.allocated().values()))
        self.nc.all_engine_barrier()


def build_bass(n_cores=N_CORES, collective=True, fast_bn=True):
    """Build the per-core Bass module (SPMD: same program on every core).

    fast_bn: beta==0 specialization — sign(A*(z-mu)+0) == sign(gamma) *
    sign(z-mu) exactly (A = alpha*gamma*rsqrt(..) and alpha,rsqrt > 0), so
    the variance/sumsq pipeline is skipped entirely and only the per-channel
    sum is all-reduced.
    """
    nc = bass.Bass(num_devices=n_cores)

    xq_d = nc.dram_tensor("xq", [IMGS, CIN, XLEN], F8, kind="ExternalInput")
    ws_d = nc.dram_tensor("ws", [CIN, NCHUNK * NPAIR * 2 * 128], F8,
                          kind="ExternalInput")
    abg_d = nc.dram_tensor("abg", [128, 3 * NCHUNK], F32, kind="ExternalInput")
    out_d = nc.dram_tensor("out", [IMGS, NCHUNK, 128, PIX], F8,
                           kind="ExternalOutput")
    # tail signs offloaded to DVE (bf16 bit trick): img0 full + img1 1st half
    out2_d = nc.dram_tensor("out2", [2, 128, PIX], BF16, kind="ExternalOutput")

    with _SplitDrainTileContext(nc) as tc:
        with (
            tc.tile_pool(name="const", bufs=1) as constp,
            tc.tile_pool(name="xbuf", bufs=1) as xp,
            tc.tile_pool(name="zbuf", bufs=1) as zp,
            tc.tile_pool(name="stats", bufs=1) as sp,
            tc.tile_pool(name="sq", bufs=2) as sqp,
            tc.tile_pool(name="ostg", bufs=4) as op_,
            tc.tile_pool(name="pz", bufs=8, space="PSUM") as pp,
            tc.tile_pool(name="dram", bufs=1, space="DRAM") as dp,
        ):
            # ---- constants (weights split per chunk so chunk 0 loads fast) ----
            w_sb = [constp.tile([128, NPAIR * 2 * 128], F8, tag=f"wpk{j}",
                                name=f"wpk{j}") for j in range(NCHUNK)]
            abg_sb = constp.tile([128, 3 * NCHUNK], F32, tag="abg")
            wv = ws_d[:].rearrange("p (j r) -> p j r", j=NCHUNK)
            nc.sync.dma_start(w_sb[0][:, : 7 * 256], wv[:, 0, : 7 * 256])

            # ---- x tiles; image 0 loads per-plane so PE starts early ----
            xt = []
            for img in range(IMGS):
                t = xp.tile([128, XLEN], F8, tag=f"x{img}", name=f"x{img}")
                xt.append(t)
            # img0 loads in contiguous row-range pieces (planes interleaved
            # per row) so unit rt=0 starts once rows 0..10 have landed
            for r0, r1 in ((0, 10), (10, 26), (26, HP)):
                o = r0 * ROWSTR
                e = r1 * ROWSTR if r1 < HP else XLEN
                nc.sync.dma_start(xt[0][:, o:e], xq_d[0][:, o:e])
                if r1 == 10:   # rest of chunk-0 weights after the first piece
                    nc.sync.dma_start(w_sb[0][:, 7 * 256 :],
                                      wv[:, 0, 7 * 256 :])
            nc.sync.dma_start(w_sb[1][:], wv[:, 1])
            for img in range(1, IMGS):
                nc.sync.dma_start(xt[img][:], xq_d[img])
            nc.sync.dma_start(abg_sb[:], abg_d[:])

            # ---- z buffers + stats ----
            z = [zp.tile([128, IMGS * PIX], F32, tag=f"z{j}", name=f"z{j}")
                 for j in range(NCHUNK)]
            ssum = sp.tile([128, 64], F32, tag="ssum")
            ssq = None if fast_bn else sp.tile([128, 64], F32, tag="ssq")

            # host-precomputed columns (see _prep_inputs):
            #  fast_bn: ag=sign(gamma), na2=-sign(gamma), be unused
            #  general: ag=alpha*gamma, na2=-alpha^2,     be=beta
            ag = abg_sb[:, 0:NCHUNK]
            na2 = abg_sb[:, NCHUNK : 2 * NCHUNK]
            be = abg_sb[:, 2 * NCHUNK : 3 * NCHUNK]
            inv_n = 1.0 / NTOT
            npart = IMGS * RT

            AB = {}

            def emit_sign(j, img, lo, hi):
                """sign(z*A+B) for pixels [lo,hi) of (chunk j, img) -> DRAM."""
                A, B = AB[j]
                ostg = op_.tile([128, hi - lo], F8, tag="ostg",
                                name=f"ostg{j}_{img}_{lo}")
                nc.scalar.activation(
                    out=ostg[:], in_=z[j][:, img * PIX + lo : img * PIX + hi],
                    func=mybir.ActivationFunctionType.Sign,
                    bias=B[:, 0:1], scale=A[:, 0:1],
                )
                nc.sync.dma_start(out_d[img, j][:, lo:hi], ostg[:])

            # chunk-0 sign work, split into quarter-images, interleaved into
            # chunk-1's conv loop so the ACT engine never bursts
            c0_pieces = [(img, lo, lo + PIX // 4)
                         for img in range(IMGS)
                         for lo in range(0, PIX, PIX // 4)]

            # Per chunk: conv -> stats AllReduce -> sign+store. Chunk 0's
            # collective + BN tail overlaps chunk 1's conv on PE.
            for j in range(NCHUNK):
                unit = 0
                for img in range(IMGS):
                    xa = xt[img][:]
                    part_dim = list(xa.ap[0])
                    for rt in range(RT):
                        pt = pp.tile([128, NVAL], F32, tag="pz",
                                     name=f"pz{j}_{img}_{rt}")
                        for q, (ta, tb) in enumerate(PAIRS):
                            off_a = _tap_off(ta[0], ta[1], rt)
                            if tb is None:
                                delta = 1          # zero weights; any finite data
                            else:
                                delta = _tap_off(tb[0], tb[1], rt) - off_a
                            # moving free = [pair, row, col]; only the 448
                            # valid pixels are computed
                            rhs = bass.AP(
                                xa.tensor, xa.offset + off_a,
                                [part_dim, [delta, 2], [ROWSTR, RTR], [1, W]],
                            )
                            woff = q * 256
                            lhsT = w_sb[j][:, woff : woff + 256].rearrange(
                                "p (two m) -> p two m", two=2
                            )
                            nc.tensor.matmul(
                                pt[:], lhsT, rhs,
                                start=(q == 0), stop=(q == NPAIR - 1),
                                perf_mode=mybir.MatmulPerfMode.DoubleRow,
                            )
                        col = img * RT + rt
                        zs = z[j][:, img * PIX + rt * NVAL
                                  : img * PIX + (rt + 1) * NVAL]
                        nc.vector.tensor_scalar(
                            out=zs, in0=pt[:], scalar1=0.0, scalar2=None,
                            op0=mybir.AluOpType.add, op1=mybir.AluOpType.add,
                            accum_out=ssum[:, j * npart + col
                                           : j * npart + col + 1],
                        )
                        if not fast_bn:
                            sqt = sqp.tile([128, NVAL], F32, tag="sqt")
                            nc.scalar.activation(
                                out=sqt[:], in_=pt[:],
                                func=mybir.ActivationFunctionType.Square,
                                accum_out=ssq[:, j * npart + col
                                              : j * npart + col + 1],
                            )
                        if j == 1 and unit < len(c0_pieces):
                            emit_sign(0, *c0_pieces[unit])
                        unit += 1

                # ---- chunk-j stats: [128,SW] = (sum[, sumsq]) ----
                SW = 1 if fast_bn else 2
                cc_sb = sp.tile([128, SW], F32, tag=f"ccsb{j}", name=f"ccsb{j}")
                nc.vector.reduce_sum(
                    out=cc_sb[:, 0:1], in_=ssum[:, j * npart : (j + 1) * npart],
                    axis=mybir.AxisListType.X,
                )
                if not fast_bn:
                    nc.vector.reduce_sum(
                        out=cc_sb[:, 1:2],
                        in_=ssq[:, j * npart : (j + 1) * npart],
                        axis=mybir.AxisListType.X,
                    )
                st = sp.tile([128, SW], F32, tag=f"st{j}", name=f"st{j}")
                if collective and n_cores > 1:
                    cc_in = dp.tile([128, SW], F32, tag=f"ccin{j}",
                                    name=f"ccin{j}")
                    cc_out = dp.tile([128, SW], F32, tag=f"ccout{j}",
                                     name=f"ccout{j}")
                    nc.sync.dma_start(cc_in[:], cc_sb[:])
                    nc.gpsimd.collective_compute(
                        "AllReduce", mybir.AluOpType.add,
                        replica_groups=[list(range(n_cores))],
                        ins=[cc_in.opt()], outs=[cc_out.opt()],
                    )
                    nc.sync.dma_start(st[:], cc_out[:])
                else:
                    nc.vector.tensor_copy(st[:], cc_sb[:])

                B = sp.tile([128, 1], F32, tag=f"B{j}", name=f"B{j}")
                if fast_bn:
                    # beta == 0: sign(A*(z-mu)) == sign(gamma)*sign(z-mu);
                    # abg carries sg=sign(gamma) and nsg=-sign(gamma).
                    # A = sg (host constant);  B = mu*nsg = -mu*sg
                    A = ag[:, j : j + 1]
                    nc.vector.tensor_scalar(
                        out=B[:], in0=st[:, 0:1], scalar1=inv_n,
                        scalar2=na2[:, j : j + 1],
                        op0=mybir.AluOpType.mult, op1=mybir.AluOpType.mult)
                else:
                    A = sp.tile([128, 1], F32, tag=f"A{j}", name=f"A{j}")
                    # ms=(mu,m2); nv=mu^2-m2=-var; v2=nv*(-a2)+eps;
                    # A = ag/sqrt(v2); B = beta - mu*A
                    ms = sp.tile([128, 2], F32, tag=f"ms{j}", name=f"ms{j}")
                    nv = sp.tile([128, 1], F32, tag=f"nv{j}", name=f"nv{j}")
                    tmp = sp.tile([128, 1], F32, tag=f"tmp{j}", name=f"tmp{j}")
                    nc.vector.tensor_scalar(out=ms[:], in0=st[:],
                                            scalar1=inv_n, scalar2=None,
                                            op0=mybir.AluOpType.mult)
                    nc.vector.scalar_tensor_tensor(
                        out=nv[:], in0=ms[:, 0:1], scalar=ms[:, 0:1],
                        in1=ms[:, 1:2], op0=mybir.AluOpType.mult,
                        op1=mybir.AluOpType.subtract)
                    nc.vector.tensor_scalar(
                        out=tmp[:], in0=nv[:], scalar1=na2[:, j : j + 1],
                        scalar2=float(BN_EPS), op0=mybir.AluOpType.mult,
                        op1=mybir.AluOpType.add)
                    nc.scalar.sqrt(tmp[:], tmp[:])
                    nc.vector.reciprocal(tmp[:], tmp[:])  # rsqrt(a^2 var+eps)
                    nc.vector.tensor_scalar(out=A[:], in0=tmp[:],
                                            scalar1=ag[:, j : j + 1],
                                            scalar2=None,
                                            op0=mybir.AluOpType.mult)
                    nc.vector.tensor_tensor(out=tmp[:], in0=ms[:, 0:1],
                                            in1=A[:], op=mybir.AluOpType.mult)
                    nc.vector.tensor_tensor(out=B[:], in0=be[:, j : j + 1],
                                            in1=tmp[:],
                                            op=mybir.AluOpType.subtract)
                AB[j] = (A, B)

                # chunk-1 signs run in the tail, split ACT/DVE.  DVE bit
                # trick: sign(y) == (y & 0x8000) | 0x3f80 on the bf16
                # encoding (rounding y to bf16 preserves its sign exactly).
                if j == NCHUNK - 1:
                    def dve_sign(row, img, lo, hi):
                        n = hi - lo
                        ybf = sqp.tile([128, n], BF16, tag="ybf",
                                       name=f"ybf{row}")
                        nc.vector.tensor_scalar(
                            out=ybf[:],
                            in0=z[j][:, img * PIX + lo : img * PIX + hi],
                            scalar1=A[:, 0:1], scalar2=B[:, 0:1],
                            op0=mybir.AluOpType.mult, op1=mybir.AluOpType.add,
                        )
                        osg2 = op_.tile([128, n], BF16, tag="osg2",
                                        name=f"osg2_{row}")
                        nc.vector.tensor_scalar(
                            out=osg2[:].bitcast(mybir.dt.int16),
                            in0=ybf[:].bitcast(mybir.dt.int16),
                            scalar1=0x8000, scalar2=0x3F80,
                            op0=mybir.AluOpType.bitwise_and,
                            op1=mybir.AluOpType.bitwise_or,
                        )
                        nc.sync.dma_start(out2_d[row][:, lo:hi], osg2[:])

                    dve_sign(0, 0, 0, PIX)             # img0 full on DVE
                    dve_sign(1, 1, 0, PIX // 2)        # img1 1st half on DVE
                    emit_sign(j, 2, 0, PIX)            # ACT
                    emit_sign(j, 3, 0, PIX)            # ACT
                    emit_sign(j, 1, PIX // 2, PIX)     # ACT, small flush last

    _split_multi_waits(nc)
    return nc


def _prep_inputs(x, weight, gamma, beta, fast_bn=True):
    """Host-side prep: alpha/sign folding, padding, fp8 hi/mid/lo split."""
    x = np.ascontiguousarray(x, dtype=np.float32)
    weight = np.ascontiguousarray(weight, dtype=np.float32)

    alpha = np.abs(weight).mean(axis=(1, 2, 3)).astype(np.float32)      # [256]
    sgn = np.where(weight >= 0, np.float32(1), np.float32(-1))          # [256,128,3,3]

    # DoubleRow-packed weights: wpk[cin, chunk, pair, half, m] = sgn * scale
    wpk = np.zeros((CIN, NCHUNK, NPAIR, 2, 128), np.float32)
    for j in range(NCHUNK):
        for q, (ta, tb) in enumerate(PAIRS):
            for h, t in ((0, ta), (1, tb)):
                if t is None:
                    continue
                plane, k = t
                dy, dx = divmod(k, 3)
                wpk[:, j, q, h, :] = (
                    sgn[j * 128 : (j + 1) * 128, :, dy, dx].T * SCALES[plane]
                )
    ws = np.ascontiguousarray(
        wpk.reshape(CIN, NCHUNK * NPAIR * 2 * 128)
    ).astype(E4M3)

    # abg[p, j] columns (channel o = j*128+p):
    #  fast_bn: [sign(gamma) | -sign(gamma) | 0]
    #  general: [alpha*gamma | -alpha^2    | beta]
    def chunked(v):
        return np.ascontiguousarray(v.reshape(NCHUNK, 128).T)  # [128, 2]
    gamma = np.asarray(gamma, np.float32)
    if fast_bn:
        sg = np.where(gamma >= 0, np.float32(1), np.float32(-1))
        cols = [chunked(sg), chunked(-sg), chunked(np.zeros_like(sg))]
    else:
        cols = [chunked(alpha * gamma), chunked(-alpha * alpha),
                chunked(np.asarray(beta, np.float32))]
    abg = np.concatenate(cols, axis=1).astype(np.float32)               # [128, 6]

    # fp8 hi/mid/lo split of the padded input, planes interleaved per row
    xpad = np.zeros((N_FULL, CIN, HP * WP), np.float32)
    xpad.reshape(N_FULL, CIN, HP, WP)[:, :, 1 : H + 1, 1 : W + 1] = x
    hi_q = xpad.astype(E4M3)
    r1 = xpad - hi_q.astype(np.float32)
    mid_q = (r1 * 16.0).astype(E4M3)
    r2 = r1 - mid_q.astype(np.float32) * (1.0 / 16.0)
    lo_q = (r2 * 64.0).astype(E4M3)

    xq = np.zeros((N_FULL, CIN, XLEN), E4M3)
    xv = xq[:, :, : HP * ROWSTR].reshape(N_FULL, CIN, HP, 3, WP)
    xv[:, :, :, 0, :] = hi_q.reshape(N_FULL, CIN, HP, WP)
    xv[:, :, :, 1, :] = mid_q.reshape(N_FULL, CIN, HP, WP)
    xv[:, :, :, 2, :] = lo_q.reshape(N_FULL, CIN, HP, WP)

    in_maps = []
    for c in range(N_CORES):
        sl = slice(c * IMGS, (c + 1) * IMGS)
        in_maps.append({
            "xq": np.ascontiguousarray(xq[sl]),
            "ws": ws,
            "abg": abg,
        })
    return in_maps


def kernel(x, weight, gamma, beta):
    fast_bn = bool(np.all(np.asarray(beta) == 0))
    in_maps = _prep_inputs(x, weight, gamma, beta, fast_bn=fast_bn)
    nc = build_bass(fast_bn=fast_bn)
    res = run_bass_kernel_spmd(nc, in_maps, core_ids=list(range(N_CORES)))
    out = np.empty((N_FULL, COUT, H, W), np.float32)
    for c in range(N_CORES):
        o = res.results[c]["out"].astype(np.float32)  # [IMGS,2,128,3136] +-1
        o2 = res.results[c]["out2"].astype(np.float32)  # DVE-signed parts
        o[0, 1] = o2[0]
        o[1, 1, :, : PIX // 2] = o2[1][:, : PIX // 2]
        out[c * IMGS : (c + 1) * IMGS] = o.reshape(IMGS, COUT, H, W)
    return out
